# revision 37
# baseline (speedup 1.0000x reference)
"""Trainium2 Bass kernel for the correlation-softargmax flow module.

Math (per batch b, query pixel q=(y,x)):
  c1 = l2norm_C(feature1), warp = l2norm_C(feature2)
  s[l,q] = <3x3 patch of warp at l, 3x3 patch of c1 at q>    (D = 32*9 = 288)
  p = softmax_l(10*s);  flow = (E_p[ix_l] - x, E_p[iy_l] - y)

Key factorization: the 3x3 tap sum is separable over (dy, dx) and each tap
shifts BOTH l and q:  s = sum_dx A[(ly,lx+dx),(qy,qx+dx)]  where
A = sum_{c,dy} warp[c,ly+dy,lx]*c1[c,qy+dy,qx]  (y-taps folded into one K=96
matmul).  The x-tap sum is applied AFTER the exp via
exp(a+b+c) = exp(a)*exp(b)*exp(c): E = exp(10*A - 10) and the dx=+-1 factors
are diagonal (partition+-1, free+-1) shifted copies of E, produced by DMA with
boundary cells pre-filled with exp(-10) (the "A=0 out-of-image" factor).  Two
bf16 DVE multiplies then give p = E*Em1*Ep1 = exp(10*s - 30), halving the PE
matmul work per tile (one K=96 score matmul + one stats matmul instead of
three score matmuls + stats).

Because softmax normalizes, only Z = sum_l p, Sy = sum_l p*iy, Sx = sum_l p*ix
are needed per q (flash-attention style, no [L,L] materialization; the fixed
-30 shift keeps everything in range since |10*s| <= 90).

Sharding: 8 cores = 4 batches x 2 query-row halves. Each core holds the full
K-side image (softmax runs over all 4096 l) and 2048 queries.
"""

import sys

import numpy as np

sys.path.insert(0, "/opt/trn_rl_repo")

import concourse.bass as bass  # noqa: E402
import concourse.mybir as mybir  # noqa: E402
import concourse.tile as tile  # noqa: E402
from concourse import bacc, bass_utils  # noqa: E402

F32 = mybir.dt.float32
F32R = mybir.dt.float32r
F16 = mybir.dt.float16
BF16 = mybir.dt.bfloat16

B, C, H, W = 4, 32, 64, 64
L = H * W              # 4096 match locations
NQ = L // 2            # queries per core
QROWS = H // 2         # query rows per core
NQT = NQ // 128        # epilogue bounce columns
N_CORES = 8
SCALE = 10.0
SHIFT1 = -10.0         # per-factor shift; 3 factors -> exp(10*s - 30)
EM_BORDER = float(np.exp(-10.0))   # exp(10*0 - 10): out-of-image tap factor
EPS = 1e-12

_NC_CACHE = {}
_LAST_RES = None


def _build_nc():
    nc = bacc.Bacc(None, target_bir_lowering=False)

    f1h = nc.dram_tensor("f1h", [C, QROWS + 2, W], F32, kind="ExternalInput")
    f2 = nc.dram_tensor("f2", [C, H, W], F32, kind="ExternalInput")
    w3 = nc.dram_tensor("w3", [128, 96], F32, kind="ExternalInput")
    yqt = nc.dram_tensor("yqt", [128, NQT], F32, kind="ExternalInput")
    xqt = nc.dram_tensor("xqt", [128, NQT], F32, kind="ExternalInput")
    outp = nc.dram_tensor("outp", [2, NQ], F32, kind="ExternalOutput")

    n1 = (QROWS + 2) * W   # 2176 pixels in the f1 halo slab
    n_lt = L // 128        # 32 l-tiles
    n_qt = NQ // 512       # 4 q-tiles
    DELAY = 3              # stats matmul trails its p by this many rounds

    with tile.TileContext(nc) as tc:
        with tc.tile_pool(name="big", bufs=1) as big, \
             tc.tile_pool(name="work", bufs=1) as work, \
             tc.tile_pool(name="small", bufs=1) as small, \
             tc.tile_pool(name="pp", bufs=1) as pp, \
             tc.tile_pool(name="epi", bufs=1) as epi, \
             tc.tile_pool(name="nps", bufs=2, space="PSUM") as nps, \
             tc.tile_pool(name="sps", bufs=4, space="PSUM") as sps, \
             tc.tile_pool(name="stps", bufs=2, space="PSUM") as stps:

            # ---- load inputs (images packed 4 channel-blocks deep so the
            # squares run 128 partitions wide) ----
            XS2 = L // 4           # 1024 pixels per f2 block
            XS1 = n1 // 4          # 544 pixels per f1 block
            raw2 = big.tile([128, XS2], F32, tag="raw2")
            f2f = f2.rearrange("c h w -> c (h w)")
            raw1 = big.tile([128, XS1], F32, tag="raw1")
            f1f = f1h.rearrange("c h w -> c (h w)")
            for j in range(4):
                nc.sync.dma_start(out=raw2[32 * j:32 * j + 32, :],
                                  in_=f2f[:, XS2 * j:XS2 * (j + 1)])
                nc.scalar.dma_start(out=raw1[32 * j:32 * j + 32, :],
                                    in_=f1f[:, XS1 * j:XS1 * (j + 1)])
            w3f = small.tile([128, 96], F32, tag="w3f")
            nc.scalar.dma_start(out=w3f, in_=w3[:, :])
            xqs = small.tile([128, NQT], F32, tag="xqs")
            nc.scalar.dma_start(out=xqs, in_=xqt[:, :])
            yqs = small.tile([128, NQT], F32, tag="yqs")
            nc.scalar.dma_start(out=yqs, in_=yqt[:, :])

            onesf = small.tile([128, 1], F32, tag="onesf")
            nc.vector.memset(onesf, 1.0)
            ones128 = small.tile([128, 1], F32R, tag="ones128")
            nc.scalar.copy(ones128, onesf)
            shiftc = small.tile([128, 1], F32, tag="shiftc")
            nc.vector.memset(shiftc, SHIFT1)
            eps2c = small.tile([128, 1], F32, tag="eps2c")
            nc.vector.memset(eps2c, EPS * EPS)

            # E/Em1/Ep1 live in an x-padded [128, .., 66] layout: col 0 and 65
            # of each q-row hold exp(-10) (the out-of-image tap factor).  The
            # diagonal shift then becomes ONE contiguous flat copy per
            # partition run: Em1[p, 1:] = E[p-1, :-1] — E's border columns
            # flow into exactly the right border slots of Em1, including
            # across q-row AND tile boundaries, so OCT tiles are shifted by a
            # single pair of DMAs (DMA cost is latency-dominated: ~0.6us
            # fixed per dma_start).  Persistent ring tiles; borders are
            # prefilled once and never overwritten (DMAs/exp write interiors
            # only).
            WP = W + 2
            OCT = 4        # l-tiles per shift/mul batch
            ORR = 8 * OCT  # q-rows per batch tile
            NE = 3         # E ring depth (batch granularity)
            NS = 3         # shift ring depth
            NP = 8         # p ring depth
            et = []
            em1t = []
            ep1t = []
            pt = []
            for i in range(NE):
                e = pp.tile([128, ORR, WP], BF16, tag=f"E_{i}", name=f"E_{i}")
                # only the x-border columns need the exp(-10) fill (interiors
                # are fully rewritten by exp each use)
                nc.gpsimd.memset(e[:, :, 0:1], EM_BORDER)
                nc.gpsimd.memset(e[:, :, W + 1:W + 2], EM_BORDER)
                et.append(e)
            for i in range(NS):
                em = pp.tile([128, ORR, WP], BF16, tag=f"em1_{i}",
                             name=f"em1_{i}")
                nc.gpsimd.memset(em, EM_BORDER)
                em1t.append(em)
                ep = pp.tile([128, ORR, WP], BF16, tag=f"ep1_{i}",
                             name=f"ep1_{i}")
                nc.gpsimd.memset(ep, EM_BORDER)
                ep1t.append(ep)
            for i in range(NP):
                # p/tmp are fully written every use; no prefill needed
                p_sb = pp.tile([128, ORR, WP], BF16, tag=f"p_{i}",
                               name=f"p_{i}")
                pt.append(p_sb)
            tmp = pp.tile([128, ORR, WP], BF16, tag="tmp", name="tmp")

            # ---- l2 normalization over C (C sits on partitions, so the
            # per-pixel sum of squares comes from a ones-vector matmul; the
            # 1/norm row is broadcast back across partitions with a K=1
            # ones-matmul and the scaling multiply reads it from PSUM).
            # Images are padded in y ONLY (x taps are handled post-exp). ----
            sq2 = big.tile([128, XS2], F32R, tag="sq2")
            nc.vector.tensor_mul(sq2, raw2, raw2)
            sq1 = big.tile([128, XS1], F32R, tag="sq1")
            nc.vector.tensor_mul(sq1, raw1, raw1)

            # ss chunks: (packed squares, block, offset, n, flat pixel pos)
            chunks = []
            for j in range(4):
                chunks.append((sq2, j, 0, 512, XS2 * j))
                chunks.append((sq2, j, 512, 512, XS2 * j + 512))
            for j in range(4):
                chunks.append((sq1, j, 0, 512, L + XS1 * j))
                chunks.append((sq1, j, 512, XS1 - 512, L + XS1 * j + 512))
            nrow = work.tile([1, L + n1], F32, tag="row", name="nrow")
            for ci, (src, j, o, n, pos) in enumerate(chunks):
                ssp = nps.tile([1, 512], F32, tag="nps", name="ssp")
                nc.tensor.matmul(ssp[:, :n], ones128[32 * j:32 * j + 32, :],
                                 src[32 * j:32 * j + 32, o:o + n],
                                 start=True, stop=True,
                                 tile_position=(32 * j, 0))
                # norm = sqrt(ss + eps^2) folded into the PSUM->SBUF copy
                nc.scalar.activation(nrow[:, pos:pos + n], ssp[:, :n],
                                     mybir.ActivationFunctionType.Sqrt,
                                     bias=eps2c[0:1, :])
            # constants not needed until the scale/stats phases: emitted
            # here so their vector-queue time overlaps the ss chain
            w3r = small.tile([128, 96], BF16, tag="w3r")
            nc.scalar.copy(w3r, w3f)
            onesbf = small.tile([1, C], F32, tag="onesbf")
            nc.vector.memset(onesbf, 1.0)
            onesb = small.tile([1, C], F32R, tag="onesb")
            nc.scalar.copy(onesb, onesbf)

            npx = L + n1
            # 1/sqrt runs 128 partitions wide via a [128, npx/128] bounce
            nT = work.tile([128, npx // 128], F32, tag="nT")
            nc.sync.dma_start(out=nT,
                              in_=nrow.rearrange("a (p c) -> a p c", p=128))
            rT = work.tile([128, npx // 128], F32R, tag="rT")
            with nc.allow_low_precision(reason="f32r 1/norm, 12 bits"):
                nc.vector.reciprocal(rT, nT)
            rrow = work.tile([1, npx], F32R, tag="row", name="rrow")
            nc.sync.dma_start(out=rrow.rearrange("a (p c) -> a p c", p=128),
                              in_=rT)

            # scaled, y-padded fp16 images (written through flat gap-free
            # views, so scale chunks need no image-row alignment)
            pad2 = big.tile([C, H + 2, W], F16, tag="pad2")
            nc.vector.memset(pad2[:, 0:1, :], 0.0)
            nc.vector.memset(pad2[:, H + 1:H + 2, :], 0.0)
            pad1 = big.tile([C, QROWS + 2, W], F16, tag="pad1")
            pad2f = pad2.rearrange("c h w -> c (h w)")
            pad1f = pad1.rearrange("c h w -> c (h w)")

            def scale_img(rawp, XS, base, padf, pos0):
                # chunks aligned to the packed blocks; flat-offset writes
                for j in range(4):
                    for (o, n) in ((0, 512), (512, XS - 512)):
                        rb = nps.tile([C, 512], F32, tag="nps", name="rb")
                        g = XS * j + o
                        nc.tensor.matmul(rb[:, :n], onesb,
                                         rrow[:, base + g:base + g + n],
                                         start=True, stop=True)
                        nc.vector.tensor_mul(  # gpsimd cannot read PSUM
                            padf[:, pos0 + g:pos0 + g + n],
                            rawp[32 * j:32 * j + 32, o:o + n],
                            rb[:, :n],
                        )

            scale_img(raw2, XS2, 0, pad2f, pos0=W)
            scale_img(raw1, XS1, L, pad1f, pos0=0)

            # ---- d-major y-tap patch tensors: 3 taps of 32 channels (96
            # partitions); each tap is one strided fp16 DMA copy ----
            kc = big.tile([96, H, W], F16, tag="kc")
            qc = big.tile([96, QROWS, W], F16, tag="qc")
            dma_engs = [nc.sync, nc.scalar, nc.sync]
            for j in range(3):
                dma_engs[j].dma_start(out=kc[32 * j:32 * j + 32, :, :],
                                      in_=pad2[:, j:j + H, :])
                dma_engs[j].dma_start(out=qc[32 * j:32 * j + 32, :, :],
                                      in_=pad1[:, j:j + QROWS, :])

            # ---- main loop: A-matmul -> E=exp -> diag shifts -> p -> stats,
            # flash-attention style over l ----
            n_rounds = n_qt * n_lt
            DELAY = 6 * OCT    # stats matmul trails its p by six batches
            stats_t = [None] * n_qt

            def stats_mm(r):
                qt, lt = divmod(r, n_lt)
                p_oct = pt[(r // OCT) % NP]
                i = r % OCT
                nc.tensor.matmul(stats_t[qt], w3r[:, 3 * lt:3 * lt + 3],
                                 p_oct[:, 8 * i:8 * i + 8, 1:W + 1],
                                 start=(lt == 0), stop=(lt == n_lt - 1))
                if lt == n_lt - 1:
                    nc.scalar.copy(stats_sb[:, 512 * qt:512 * (qt + 1)],
                                   stats_t[qt])

            stats_sb = epi.tile([3, NQ], F32, tag="stats_sb", bufs=1)

            for r in range(n_rounds):
                qt, lt = divmod(r, n_lt)
                if lt == 0:
                    stats_t[qt] = stps.tile([3, 512], F32, tag="stats",
                                            name="stats")
                psA = sps.tile([128, 512], F32, tag="A", name="psA")
                nc.tensor.matmul(psA, kc[:, 2 * lt:2 * lt + 2, :],
                                 qc[:, 8 * qt:8 * qt + 8, :],
                                 start=True, stop=True)
                if r >= DELAY:
                    stats_mm(r - DELAY)
                e_sb = et[(r // OCT) % NE]
                i = r % OCT
                nc.scalar.activation(e_sb[:, 8 * i:8 * i + 8, 1:W + 1], psA,
                                     mybir.ActivationFunctionType.Exp,
                                     bias=shiftc, scale=SCALE)
                if i == OCT - 1:
                    o = r // OCT
                    em1 = em1t[o % NS]
                    ep1 = ep1t[o % NS]
                    nf = ORR * WP
                    ef = e_sb.rearrange("p a b -> p (a b)")
                    emf = em1.rearrange("p a b -> p (a b)")
                    epf = ep1.rearrange("p a b -> p (a b)")
                    # diagonal shifts: +1 in both lx (partition) and qx
                    # (free), ONE contiguous DMA per shift on the two HWDGE
                    # queues; the run crosses the lx=0/63 border at one
                    # partition (em1 p=64 / ep1 p=63), repaired by a small
                    # gpsimd memset back to the exp(-10) border value
                    nc.sync.dma_start(out=emf[1:64, 1:nf],
                                      in_=ef[0:63, 0:nf - 1])
                    nc.sync.dma_start(out=emf[65:128, 1:nf],
                                      in_=ef[64:127, 0:nf - 1])
                    nc.gpsimd.dma_start(out=epf[0:63, 0:nf - 1],
                                        in_=ef[1:64, 1:nf])
                    nc.gpsimd.dma_start(out=epf[64:127, 0:nf - 1],
                                        in_=ef[65:128, 1:nf])
                    p_oct = pt[o % NP]
                    nc.vector.tensor_mul(tmp, em1, ep1)
                    nc.vector.tensor_mul(p_oct, tmp, e_sb)
            for r in range(n_rounds - DELAY, n_rounds):
                stats_mm(r)

            # ---- epilogue: flow = S/Z - coord, bounced to [128, NQT] so the
            # reciprocal runs 128-wide ----
            zT = epi.tile([128, NQT], F32, tag="zT")
            nc.sync.dma_start(out=zT,
                              in_=stats_sb[0:1, :].rearrange("a (p c) -> a p c", p=128))
            syT = epi.tile([128, NQT], F32, tag="syT")
            nc.sync.dma_start(out=syT,
                              in_=stats_sb[1:2, :].rearrange("a (p c) -> a p c", p=128))
            sxT = epi.tile([128, NQT], F32, tag="sxT")
            nc.sync.dma_start(out=sxT,
                              in_=stats_sb[2:3, :].rearrange("a (p c) -> a p c", p=128))
            rz = epi.tile([128, NQT], F32R, tag="rz")
            with nc.allow_low_precision(reason="f32r 1/Z, 12 bits"):
                nc.vector.reciprocal(rz, zT)
            fw = epi.tile([128, NQT], F32, tag="fw")
            nc.vector.tensor_mul(fw, sxT, rz)
            nc.vector.tensor_sub(fw, fw, xqs)
            fh = epi.tile([128, NQT], F32, tag="fh")
            nc.vector.tensor_mul(fh, syT, rz)
            nc.vector.tensor_sub(fh, fh, yqs)
            nc.sync.dma_start(
                out=outp[0:1, :].rearrange("a (p c) -> a p c", p=128), in_=fw)
            nc.sync.dma_start(
                out=outp[1:2, :].rearrange("a (p c) -> a p c", p=128), in_=fh)

    nc.finalize()
    return nc


def _host_consts():
    p = np.arange(128)
    w3 = np.zeros((128, 96), np.float32)
    for t in range(32):
        w3[:, 3 * t] = 1.0
        w3[:, 3 * t + 1] = 2 * t + p // 64   # global iy of l = 128*lt + p
        w3[:, 3 * t + 2] = p % 64            # global ix
    # epilogue bounce layout: q = p*NQT + c  ->  [p, c]
    q = np.arange(NQ).reshape(128, NQT)      # [128, NQT], q = NQT*p + c
    xq = (q % W).astype(np.float32)
    ly = (q // W).astype(np.float32)
    return w3, xq, ly


def kernel(feature1, feature2):
    feature1 = np.ascontiguousarray(feature1, np.float32)
    feature2 = np.ascontiguousarray(feature2, np.float32)
    w3, xq, ly = _host_consts()

    f1p = np.zeros((B, C, H + 2, W), np.float32)
    f1p[:, :, 1:H + 1, :] = feature1

    in_maps = []
    for core in range(N_CORES):
        b, h = divmod(core, 2)
        in_maps.append({
            "f1h": np.ascontiguousarray(f1p[b, :, h * QROWS:h * QROWS + QROWS + 2, :]),
            "f2": np.ascontiguousarray(feature2[b]),
            "w3": w3,
            "yqt": ly + h * QROWS,
            "xqt": xq,
        })

    if "nc" not in _NC_CACHE:
        _NC_CACHE["nc"] = _build_nc()
    res = bass_utils.run_bass_kernel_spmd(
        _NC_CACHE["nc"], in_maps, core_ids=list(range(N_CORES)))
    global _LAST_RES
    _LAST_RES = res

    out = np.zeros((B, 2, H, W), np.float32)
    for core in range(N_CORES):
        b, h = divmod(core, 2)
        out[b, :, h * QROWS:(h + 1) * QROWS, :] = (
            res.results[core]["outp"].reshape(2, QROWS, W))
    return out


# revision 38
# speedup vs baseline: 1.0035x; 1.0035x over previous
"""Trainium2 Bass kernel for the correlation-softargmax flow module.

Math (per batch b, query pixel q=(y,x)):
  c1 = l2norm_C(feature1), warp = l2norm_C(feature2)
  s[l,q] = <3x3 patch of warp at l, 3x3 patch of c1 at q>    (D = 32*9 = 288)
  p = softmax_l(10*s);  flow = (E_p[ix_l] - x, E_p[iy_l] - y)

Key factorization: the 3x3 tap sum is separable over (dy, dx) and each tap
shifts BOTH l and q:  s = sum_dx A[(ly,lx+dx),(qy,qx+dx)]  where
A = sum_{c,dy} warp[c,ly+dy,lx]*c1[c,qy+dy,qx]  (y-taps folded into one K=96
matmul).  The x-tap sum is applied AFTER the exp via
exp(a+b+c) = exp(a)*exp(b)*exp(c): E = exp(10*A - 10) and the dx=+-1 factors
are diagonal (partition+-1, free+-1) shifted copies of E, produced by DMA with
boundary cells pre-filled with exp(-10) (the "A=0 out-of-image" factor).  Two
bf16 DVE multiplies then give p = E*Em1*Ep1 = exp(10*s - 30), halving the PE
matmul work per tile (one K=96 score matmul + one stats matmul instead of
three score matmuls + stats).

Because softmax normalizes, only Z = sum_l p, Sy = sum_l p*iy, Sx = sum_l p*ix
are needed per q (flash-attention style, no [L,L] materialization; the fixed
-30 shift keeps everything in range since |10*s| <= 90).

Sharding: 8 cores = 4 batches x 2 query-row halves. Each core holds the full
K-side image (softmax runs over all 4096 l) and 2048 queries.
"""

import sys

import numpy as np

sys.path.insert(0, "/opt/trn_rl_repo")

import concourse.bass as bass  # noqa: E402
import concourse.mybir as mybir  # noqa: E402
import concourse.tile as tile  # noqa: E402
from concourse import bacc, bass_utils  # noqa: E402

F32 = mybir.dt.float32
F32R = mybir.dt.float32r
F16 = mybir.dt.float16
BF16 = mybir.dt.bfloat16

B, C, H, W = 4, 32, 64, 64
L = H * W              # 4096 match locations
NQ = L // 2            # queries per core
QROWS = H // 2         # query rows per core
NQT = NQ // 128        # epilogue bounce columns
N_CORES = 8
SCALE = 10.0
SHIFT1 = -10.0         # per-factor shift; 3 factors -> exp(10*s - 30)
EM_BORDER = float(np.exp(-10.0))   # exp(10*0 - 10): out-of-image tap factor
EPS = 1e-12

_NC_CACHE = {}
_LAST_RES = None


def _build_nc():
    nc = bacc.Bacc(None, target_bir_lowering=False)

    f1h = nc.dram_tensor("f1h", [C, QROWS + 2, W], F32, kind="ExternalInput")
    f2 = nc.dram_tensor("f2", [C, H, W], F32, kind="ExternalInput")
    w3 = nc.dram_tensor("w3", [128, 96], F32, kind="ExternalInput")
    yqt = nc.dram_tensor("yqt", [128, NQT], F32, kind="ExternalInput")
    xqt = nc.dram_tensor("xqt", [128, NQT], F32, kind="ExternalInput")
    outp = nc.dram_tensor("outp", [2, NQ], F32, kind="ExternalOutput")

    n1 = (QROWS + 2) * W   # 2176 pixels in the f1 halo slab
    n_lt = L // 128        # 32 l-tiles
    n_qt = NQ // 512       # 4 q-tiles
    DELAY = 3              # stats matmul trails its p by this many rounds

    with tile.TileContext(nc) as tc:
        with tc.tile_pool(name="big", bufs=1) as big, \
             tc.tile_pool(name="work", bufs=1) as work, \
             tc.tile_pool(name="small", bufs=1) as small, \
             tc.tile_pool(name="pp", bufs=1) as pp, \
             tc.tile_pool(name="epi", bufs=1) as epi, \
             tc.tile_pool(name="nps", bufs=2, space="PSUM") as nps, \
             tc.tile_pool(name="sps", bufs=4, space="PSUM") as sps, \
             tc.tile_pool(name="stps", bufs=2, space="PSUM") as stps:

            # ---- load inputs (images packed 4 channel-blocks deep so the
            # squares run 128 partitions wide) ----
            XS2 = L // 4           # 1024 pixels per f2 block
            XS1 = n1 // 4          # 544 pixels per f1 block
            raw2 = big.tile([128, XS2], F32, tag="raw2")
            f2f = f2.rearrange("c h w -> c (h w)")
            raw1 = big.tile([128, XS1], F32, tag="raw1")
            f1f = f1h.rearrange("c h w -> c (h w)")
            for j in range(4):
                nc.sync.dma_start(out=raw2[32 * j:32 * j + 32, :],
                                  in_=f2f[:, XS2 * j:XS2 * (j + 1)])
                nc.gpsimd.dma_start(out=raw1[32 * j:32 * j + 32, :],
                                    in_=f1f[:, XS1 * j:XS1 * (j + 1)])
            w3f = small.tile([128, 96], F32, tag="w3f")
            nc.scalar.dma_start(out=w3f, in_=w3[:, :])
            xqs = small.tile([128, NQT], F32, tag="xqs")
            nc.scalar.dma_start(out=xqs, in_=xqt[:, :])
            yqs = small.tile([128, NQT], F32, tag="yqs")
            nc.scalar.dma_start(out=yqs, in_=yqt[:, :])

            onesf = small.tile([128, 1], F32, tag="onesf")
            nc.vector.memset(onesf, 1.0)
            ones128 = small.tile([128, 1], F32R, tag="ones128")
            nc.scalar.copy(ones128, onesf)
            shiftc = small.tile([128, 1], F32, tag="shiftc")
            nc.vector.memset(shiftc, SHIFT1)
            eps2c = small.tile([128, 1], F32, tag="eps2c")
            nc.vector.memset(eps2c, EPS * EPS)

            # E/Em1/Ep1 live in an x-padded [128, .., 66] layout: col 0 and 65
            # of each q-row hold exp(-10) (the out-of-image tap factor).  The
            # diagonal shift then becomes ONE contiguous flat copy per
            # partition run: Em1[p, 1:] = E[p-1, :-1] — E's border columns
            # flow into exactly the right border slots of Em1, including
            # across q-row AND tile boundaries, so OCT tiles are shifted by a
            # single pair of DMAs (DMA cost is latency-dominated: ~0.6us
            # fixed per dma_start).  Persistent ring tiles; borders are
            # prefilled once and never overwritten (DMAs/exp write interiors
            # only).
            WP = W + 2
            OCT = 4        # l-tiles per shift/mul batch
            ORR = 8 * OCT  # q-rows per batch tile
            NE = 3         # E ring depth (batch granularity)
            NS = 3         # shift ring depth
            NP = 8         # p ring depth
            et = []
            em1t = []
            ep1t = []
            pt = []
            for i in range(NE):
                e = pp.tile([128, ORR, WP], BF16, tag=f"E_{i}", name=f"E_{i}")
                # only the x-border columns need the exp(-10) fill (interiors
                # are fully rewritten by exp each use)
                nc.gpsimd.memset(e[:, :, 0:1], EM_BORDER)
                nc.gpsimd.memset(e[:, :, W + 1:W + 2], EM_BORDER)
                et.append(e)
            for i in range(NS):
                em = pp.tile([128, ORR, WP], BF16, tag=f"em1_{i}",
                             name=f"em1_{i}")
                nc.gpsimd.memset(em, EM_BORDER)
                em1t.append(em)
                ep = pp.tile([128, ORR, WP], BF16, tag=f"ep1_{i}",
                             name=f"ep1_{i}")
                nc.gpsimd.memset(ep, EM_BORDER)
                ep1t.append(ep)
            for i in range(NP):
                # p/tmp are fully written every use; no prefill needed
                p_sb = pp.tile([128, ORR, WP], BF16, tag=f"p_{i}",
                               name=f"p_{i}")
                pt.append(p_sb)
            tmp = pp.tile([128, ORR, WP], BF16, tag="tmp", name="tmp")

            # ---- l2 normalization over C (C sits on partitions, so the
            # per-pixel sum of squares comes from a ones-vector matmul; the
            # 1/norm row is broadcast back across partitions with a K=1
            # ones-matmul and the scaling multiply reads it from PSUM).
            # Images are padded in y ONLY (x taps are handled post-exp). ----
            sq2 = big.tile([128, XS2], F32R, tag="sq2")
            nc.vector.tensor_mul(sq2, raw2, raw2)
            sq1 = big.tile([128, XS1], F32R, tag="sq1")
            nc.vector.tensor_mul(sq1, raw1, raw1)

            # ss chunks: (packed squares, block, offset, n, flat pixel pos)
            chunks = []
            for j in range(4):
                chunks.append((sq2, j, 0, 512, XS2 * j))
                chunks.append((sq2, j, 512, 512, XS2 * j + 512))
            for j in range(4):
                chunks.append((sq1, j, 0, 512, L + XS1 * j))
                chunks.append((sq1, j, 512, XS1 - 512, L + XS1 * j + 512))
            nrow = work.tile([1, L + n1], F32, tag="row", name="nrow")
            for ci, (src, j, o, n, pos) in enumerate(chunks):
                ssp = nps.tile([1, 512], F32, tag="nps", name="ssp")
                nc.tensor.matmul(ssp[:, :n], ones128[32 * j:32 * j + 32, :],
                                 src[32 * j:32 * j + 32, o:o + n],
                                 start=True, stop=True,
                                 tile_position=(32 * j, 0))
                # norm = sqrt(ss + eps^2) folded into the PSUM->SBUF copy
                nc.scalar.activation(nrow[:, pos:pos + n], ssp[:, :n],
                                     mybir.ActivationFunctionType.Sqrt,
                                     bias=eps2c[0:1, :])
            # constants not needed until the scale/stats phases: emitted
            # here so their vector-queue time overlaps the ss chain
            w3r = small.tile([128, 96], BF16, tag="w3r")
            nc.scalar.copy(w3r, w3f)
            onesbf = small.tile([1, C], F32, tag="onesbf")
            nc.vector.memset(onesbf, 1.0)
            onesb = small.tile([1, C], F32R, tag="onesb")
            nc.scalar.copy(onesb, onesbf)

            npx = L + n1
            # 1/sqrt runs 128 partitions wide via a [128, npx/128] bounce
            nT = work.tile([128, npx // 128], F32, tag="nT")
            nc.sync.dma_start(out=nT,
                              in_=nrow.rearrange("a (p c) -> a p c", p=128))
            rT = work.tile([128, npx // 128], F32R, tag="rT")
            with nc.allow_low_precision(reason="f32r 1/norm, 12 bits"):
                nc.vector.reciprocal(rT, nT)
            rrow = work.tile([1, npx], F32R, tag="row", name="rrow")
            nc.sync.dma_start(out=rrow.rearrange("a (p c) -> a p c", p=128),
                              in_=rT)

            # scaled, y-padded fp16 images (written through flat gap-free
            # views, so scale chunks need no image-row alignment)
            pad2 = big.tile([C, H + 2, W], F16, tag="pad2")
            nc.vector.memset(pad2[:, 0:1, :], 0.0)
            nc.vector.memset(pad2[:, H + 1:H + 2, :], 0.0)
            pad1 = big.tile([C, QROWS + 2, W], F16, tag="pad1")
            pad2f = pad2.rearrange("c h w -> c (h w)")
            pad1f = pad1.rearrange("c h w -> c (h w)")

            def scale_img(rawp, XS, base, padf, pos0):
                # chunks aligned to the packed blocks; flat-offset writes
                for j in range(4):
                    for (o, n) in ((0, 512), (512, XS - 512)):
                        rb = nps.tile([C, 512], F32, tag="nps", name="rb")
                        g = XS * j + o
                        nc.tensor.matmul(rb[:, :n], onesb,
                                         rrow[:, base + g:base + g + n],
                                         start=True, stop=True)
                        nc.vector.tensor_mul(  # gpsimd cannot read PSUM
                            padf[:, pos0 + g:pos0 + g + n],
                            rawp[32 * j:32 * j + 32, o:o + n],
                            rb[:, :n],
                        )

            scale_img(raw2, XS2, 0, pad2f, pos0=W)
            scale_img(raw1, XS1, L, pad1f, pos0=0)

            # ---- d-major y-tap patch tensors: 3 taps of 32 channels (96
            # partitions); each tap is one strided fp16 DMA copy ----
            kc = big.tile([96, H, W], F16, tag="kc")
            qc = big.tile([96, QROWS, W], F16, tag="qc")
            dma_engs = [nc.sync, nc.scalar, nc.sync]
            for j in range(3):
                dma_engs[j].dma_start(out=kc[32 * j:32 * j + 32, :, :],
                                      in_=pad2[:, j:j + H, :])
                dma_engs[j].dma_start(out=qc[32 * j:32 * j + 32, :, :],
                                      in_=pad1[:, j:j + QROWS, :])

            # ---- main loop: A-matmul -> E=exp -> diag shifts -> p -> stats,
            # flash-attention style over l ----
            n_rounds = n_qt * n_lt
            DELAY = 6 * OCT    # stats matmul trails its p by six batches
            stats_t = [None] * n_qt

            def stats_mm(r):
                qt, lt = divmod(r, n_lt)
                p_oct = pt[(r // OCT) % NP]
                i = r % OCT
                nc.tensor.matmul(stats_t[qt], w3r[:, 3 * lt:3 * lt + 3],
                                 p_oct[:, 8 * i:8 * i + 8, 1:W + 1],
                                 start=(lt == 0), stop=(lt == n_lt - 1))
                if lt == n_lt - 1:
                    nc.scalar.copy(stats_sb[:, 512 * qt:512 * (qt + 1)],
                                   stats_t[qt])

            stats_sb = epi.tile([3, NQ], F32, tag="stats_sb", bufs=1)

            for r in range(n_rounds):
                qt, lt = divmod(r, n_lt)
                if lt == 0:
                    stats_t[qt] = stps.tile([3, 512], F32, tag="stats",
                                            name="stats")
                psA = sps.tile([128, 512], F32, tag="A", name="psA")
                nc.tensor.matmul(psA, kc[:, 2 * lt:2 * lt + 2, :],
                                 qc[:, 8 * qt:8 * qt + 8, :],
                                 start=True, stop=True)
                if r >= DELAY:
                    stats_mm(r - DELAY)
                e_sb = et[(r // OCT) % NE]
                i = r % OCT
                nc.scalar.activation(e_sb[:, 8 * i:8 * i + 8, 1:W + 1], psA,
                                     mybir.ActivationFunctionType.Exp,
                                     bias=shiftc, scale=SCALE)
                if i == OCT - 1:
                    o = r // OCT
                    em1 = em1t[o % NS]
                    ep1 = ep1t[o % NS]
                    nf = ORR * WP
                    ef = e_sb.rearrange("p a b -> p (a b)")
                    emf = em1.rearrange("p a b -> p (a b)")
                    epf = ep1.rearrange("p a b -> p (a b)")
                    # diagonal shifts: +1 in both lx (partition) and qx
                    # (free), ONE contiguous DMA per shift on the two HWDGE
                    # queues; the run crosses the lx=0/63 border at one
                    # partition (em1 p=64 / ep1 p=63), repaired by a small
                    # gpsimd memset back to the exp(-10) border value
                    nc.sync.dma_start(out=emf[1:64, 1:nf],
                                      in_=ef[0:63, 0:nf - 1])
                    nc.sync.dma_start(out=emf[65:128, 1:nf],
                                      in_=ef[64:127, 0:nf - 1])
                    nc.gpsimd.dma_start(out=epf[0:63, 0:nf - 1],
                                        in_=ef[1:64, 1:nf])
                    nc.gpsimd.dma_start(out=epf[64:127, 0:nf - 1],
                                        in_=ef[65:128, 1:nf])
                    p_oct = pt[o % NP]
                    nc.vector.tensor_mul(tmp, em1, ep1)
                    nc.vector.tensor_mul(p_oct, tmp, e_sb)
            for r in range(n_rounds - DELAY, n_rounds):
                stats_mm(r)

            # ---- epilogue: flow = S/Z - coord, bounced to [128, NQT] so the
            # reciprocal runs 128-wide ----
            zT = epi.tile([128, NQT], F32, tag="zT")
            nc.sync.dma_start(out=zT,
                              in_=stats_sb[0:1, :].rearrange("a (p c) -> a p c", p=128))
            syT = epi.tile([128, NQT], F32, tag="syT")
            nc.sync.dma_start(out=syT,
                              in_=stats_sb[1:2, :].rearrange("a (p c) -> a p c", p=128))
            sxT = epi.tile([128, NQT], F32, tag="sxT")
            nc.sync.dma_start(out=sxT,
                              in_=stats_sb[2:3, :].rearrange("a (p c) -> a p c", p=128))
            rz = epi.tile([128, NQT], F32R, tag="rz")
            with nc.allow_low_precision(reason="f32r 1/Z, 12 bits"):
                nc.vector.reciprocal(rz, zT)
            fw = epi.tile([128, NQT], F32, tag="fw")
            nc.vector.tensor_mul(fw, sxT, rz)
            nc.vector.tensor_sub(fw, fw, xqs)
            fh = epi.tile([128, NQT], F32, tag="fh")
            nc.vector.tensor_mul(fh, syT, rz)
            nc.vector.tensor_sub(fh, fh, yqs)
            nc.sync.dma_start(
                out=outp[0:1, :].rearrange("a (p c) -> a p c", p=128), in_=fw)
            nc.sync.dma_start(
                out=outp[1:2, :].rearrange("a (p c) -> a p c", p=128), in_=fh)

    nc.finalize()
    return nc


def _host_consts():
    p = np.arange(128)
    w3 = np.zeros((128, 96), np.float32)
    for t in range(32):
        w3[:, 3 * t] = 1.0
        w3[:, 3 * t + 1] = 2 * t + p // 64   # global iy of l = 128*lt + p
        w3[:, 3 * t + 2] = p % 64            # global ix
    # epilogue bounce layout: q = p*NQT + c  ->  [p, c]
    q = np.arange(NQ).reshape(128, NQT)      # [128, NQT], q = NQT*p + c
    xq = (q % W).astype(np.float32)
    ly = (q // W).astype(np.float32)
    return w3, xq, ly


def kernel(feature1, feature2):
    feature1 = np.ascontiguousarray(feature1, np.float32)
    feature2 = np.ascontiguousarray(feature2, np.float32)
    w3, xq, ly = _host_consts()

    f1p = np.zeros((B, C, H + 2, W), np.float32)
    f1p[:, :, 1:H + 1, :] = feature1

    in_maps = []
    for core in range(N_CORES):
        b, h = divmod(core, 2)
        in_maps.append({
            "f1h": np.ascontiguousarray(f1p[b, :, h * QROWS:h * QROWS + QROWS + 2, :]),
            "f2": np.ascontiguousarray(feature2[b]),
            "w3": w3,
            "yqt": ly + h * QROWS,
            "xqt": xq,
        })

    if "nc" not in _NC_CACHE:
        _NC_CACHE["nc"] = _build_nc()
    res = bass_utils.run_bass_kernel_spmd(
        _NC_CACHE["nc"], in_maps, core_ids=list(range(N_CORES)))
    global _LAST_RES
    _LAST_RES = res

    out = np.zeros((B, 2, H, W), np.float32)
    for core in range(N_CORES):
        b, h = divmod(core, 2)
        out[b, :, h * QROWS:(h + 1) * QROWS, :] = (
            res.results[core]["outp"].reshape(2, QROWS, W))
    return out
